# revision 2
# baseline (speedup 1.0000x reference)
"""Expert-parallel MoE GLU FFN for Trainium2 (8 NeuronCores, Bass/Tile).

Problem: nn_ExpertFFNGroupedMM (E=8 experts, K=2, NTOK=2048, D=1024, DFF=1024,
ALIGN=16). Reference: sort routed rows by expert, pad each expert group to a
multiple of 16, grouped GEMM fc1 (GLU) -> y*silu(gate) -> grouped GEMM fc2,
scatter back to original routed-row order.

Strategy (expert parallelism, per sharding hint):
  - Host: stable-sort routed rows by expert id, compute per-expert ranks
    (exactly the reference's pad_sorted_idxs math), and build one dense token
    slab per expert, zero-padded to a common static capacity C_pad.
    This is the "all-to-all after sort" done on host.
  - Core e gets: xT (D, C_pad) tokens transposed, w1T (D, 2*DFF) = permuted
    fc1[e].T (y/gate blocks pair-interleaved so each 256-col slice is one GLU
    pair), w2T (DFF, D) = fc2[e].T.
  - Device: h = w1T.T-blocks @ x-blocks (contraction over d on the partition
    dim -> transpose-free), a = y * silu(gate), out = w2T.T-blocks @ a.
    Matmuls run as float32r (full-rate fp32 PE mode, ~1e-4 rel err).
  - Host: gather rank-rows from each expert's output and scatter to the
    original routed order.
"""

from functools import lru_cache

import numpy as np

import concourse.bacc as bacc
import concourse.tile as tile
from concourse import mybir
from concourse import bass_utils

E = 8
K = 2
NTOK = 2048
D = 1024
DFF = 1024
ALIGN = 16
N_CORES = 8

DB = D // 128     # 8 contraction blocks for fc1
JB = DFF // 128   # 8 contraction blocks for fc2
NPAIR = DFF // 128  # 8 GLU pairs (y_p, gate_p), each 128 wide

F32 = mybir.dt.float32
F32R = mybir.dt.float32r


def _plan_chunks(c: int) -> tuple[int, ...]:
    """Split padded capacity c into token chunks <=512 (fp32 moving-operand
    max / one PSUM bank), all equal and multiples of 8. float32r runs at full
    PE rate only for chunks >=256, which holds whenever c >= 512."""
    n = max(1, -(-c // 512))
    base = -(-c // (8 * n)) * 8
    return (base,) * n


@lru_cache(maxsize=4)
def _build(c_pad: int, chunks: tuple[int, ...]):
    nc = bacc.Bacc("TRN2", target_bir_lowering=False, debug=False)

    x_t = nc.dram_tensor("xT", [D, c_pad], F32R, kind="ExternalInput")
    w1_t = nc.dram_tensor("w1T", [D, 2 * DFF], F32R, kind="ExternalInput")
    w2_t = nc.dram_tensor("w2T", [DFF, D], F32R, kind="ExternalInput")
    out_t = nc.dram_tensor("outT", [D, c_pad], F32, kind="ExternalOutput")

    x3 = x_t.rearrange("(b p) c -> b p c", p=128)
    w23 = w2_t.rearrange("(b p) m -> b p m", p=128)

    with tile.TileContext(nc) as tc:
        with (
            tc.tile_pool(name="xw", bufs=1) as xw,
            tc.tile_pool(name="hps", bufs=2, space="PSUM") as hps,
            tc.tile_pool(name="ops", bufs=2, space="PSUM") as ops,
            tc.tile_pool(name="act", bufs=3) as actp,
            tc.tile_pool(name="apool", bufs=2) as apool,
            tc.tile_pool(name="outp", bufs=3) as outp,
        ):
            # Token slab: 8 d-blocks, resident for the whole kernel.
            x_sb = []
            for b in range(DB):
                t = xw.tile([128, c_pad], F32R, tag=f"x{b}", name=f"x{b}")
                nc.sync.dma_start(out=t, in_=x3[b])
                x_sb.append(t)

            # fc1 weights: one [128, 256] tile per (pair, d-block) so compute
            # on pair p can start as soon as its 8 d-slices have landed.
            w1_sb = {}
            for p in range(NPAIR):
                for b in range(DB):
                    t = xw.tile([128, 256], F32R, tag=f"w1_{p}_{b}", name=f"w1_{p}_{b}")
                    nc.sync.dma_start(
                        out=t,
                        in_=w1_t[b * 128:(b + 1) * 128, 256 * p:256 * (p + 1)],
                    )
                    w1_sb[p, b] = t

            # fc2 weights: 8 j-blocks of [128, 1024].
            w2_sb = []
            for b in range(JB):
                t = xw.tile([128, D], F32R, tag=f"w2_{b}", name=f"w2_{b}")
                nc.sync.dma_start(out=t, in_=w23[b])
                w2_sb.append(t)

            off = 0
            for ci, w in enumerate(chunks):
                a_sb = []
                for p in range(NPAIR):
                    y_ps = hps.tile([128, w], F32, tag="y", name=f"y_{ci}_{p}")
                    g_ps = hps.tile([128, w], F32, tag="g", name=f"g_{ci}_{p}")
                    for b in range(DB):
                        rhs = x_sb[b][:, off:off + w]
                        nc.tensor.matmul(
                            y_ps, w1_sb[p, b][:, 0:128], rhs,
                            start=(b == 0), stop=(b == DB - 1),
                        )
                    for b in range(DB):
                        rhs = x_sb[b][:, off:off + w]
                        nc.tensor.matmul(
                            g_ps, w1_sb[p, b][:, 128:256], rhs,
                            start=(b == 0), stop=(b == DB - 1),
                        )
                    silu = actp.tile([128, w], F32, tag="silu", name=f"s_{ci}_{p}")
                    nc.scalar.activation(
                        out=silu, in_=g_ps,
                        func=mybir.ActivationFunctionType.Silu,
                    )
                    a = apool.tile([128, w], F32R, tag=f"a{p}", name=f"a_{ci}_{p}")
                    nc.vector.tensor_mul(a, y_ps, silu)
                    a_sb.append(a)

                for mb in range(DB):
                    o_ps = ops.tile([128, w], F32, tag="o", name=f"o_{ci}_{mb}")
                    for b in range(JB):
                        nc.tensor.matmul(
                            o_ps,
                            w2_sb[b][:, mb * 128:(mb + 1) * 128],
                            a_sb[b],
                            start=(b == 0), stop=(b == JB - 1),
                        )
                    o_sb = outp.tile([128, w], F32, tag="osb", name=f"ob_{ci}_{mb}")
                    nc.vector.tensor_copy(o_sb, o_ps)
                    nc.sync.dma_start(
                        out=out_t[mb * 128:(mb + 1) * 128, off:off + w], in_=o_sb,
                    )
                off += w

    nc.compile()
    return nc


def _route(indices: np.ndarray, counts: np.ndarray):
    """Reference's sort/rank math: stable sort by expert, rank within group."""
    n = indices.size
    flat = indices.reshape(-1).astype(np.int64)
    order = np.argsort(flat, kind="stable")
    exp_sorted = flat[order]
    counts64 = counts.astype(np.int64)
    u_start = np.cumsum(counts64) - counts64
    rank = np.arange(n, dtype=np.int64) - u_start[exp_sorted]
    return order, exp_sorted, rank, counts64


def kernel(x, fc1_weight, fc2_weight, indices, counts):
    x = np.ascontiguousarray(np.asarray(x, dtype=np.float32))
    fc1_weight = np.asarray(fc1_weight, dtype=np.float32)
    fc2_weight = np.asarray(fc2_weight, dtype=np.float32)
    indices = np.asarray(indices)
    counts = np.asarray(counts)

    n = indices.size
    order, exp_sorted, rank, counts64 = _route(indices, counts)
    tok = order // K

    padded = ((counts64 + ALIGN - 1) // ALIGN) * ALIGN
    c = max(int(padded.max()), ALIGN)
    chunks = _plan_chunks(c)
    c_pad = int(sum(chunks))

    nc = _build(c_pad, chunks)

    in_maps = []
    sels = []
    for e in range(E):
        sel = exp_sorted == e
        sels.append(sel)
        r = rank[sel]
        xe = np.zeros((c_pad, D), np.float32)
        xe[r] = x[tok[sel]]
        # pair-interleave fc1 rows: (y_p, gate_p) adjacent 128-row blocks
        w1p = (
            fc1_weight[e]
            .reshape(2, NPAIR, 128, D)
            .transpose(1, 0, 2, 3)
            .reshape(2 * DFF, D)
        )
        in_maps.append({
            "xT": np.ascontiguousarray(xe.T),
            "w1T": np.ascontiguousarray(w1p.T),
            "w2T": np.ascontiguousarray(fc2_weight[e].T),
        })

    res = bass_utils.run_bass_kernel_spmd(nc, in_maps, core_ids=list(range(N_CORES)))

    out = np.zeros((n, D), np.float32)
    for e in range(E):
        sel = sels[e]
        oe = res.results[e]["outT"]  # (D, c_pad)
        out[order[sel]] = oe.T[rank[sel]]
    return out


# revision 4
# speedup vs baseline: 1.2576x; 1.2576x over previous
"""Expert-parallel MoE GLU FFN for Trainium2 (8 NeuronCores, Bass/Tile).

Problem: nn_ExpertFFNGroupedMM (E=8 experts, K=2, NTOK=2048, D=1024, DFF=1024,
ALIGN=16). Reference: sort routed rows by expert, pad each expert group to a
multiple of 16, grouped GEMM fc1 (GLU) -> y*silu(gate) -> grouped GEMM fc2,
scatter back to original routed-row order.

Strategy (expert parallelism, per the sharding hint):
  - Host: stable-sort routed rows by expert id, compute per-expert ranks
    (exactly the reference's pad_sorted_idxs math), and build one dense token
    slab per expert, zero-padded to a common static capacity C_pad.
    This is the "all-to-all after sort" done on host.
  - Host pre-packs every device input into its exact SBUF layout
    (partition-major), so each input lands in SBUF with a handful of large
    fully-contiguous DMAs (the naive strided layout left the PE starved and
    HAM-throttled behind ~100 small DMA descriptors).
  - Device (core e): h = fc1[e] @ x (contraction dim on SBUF partitions ->
    transpose-free), a = y * silu(gate), out = fc2[e] @ a. Matmuls run as
    float32r (reduced-mantissa full-rate fp32 PE mode, ~2e-4 rel err).
  - Host: gather rank-rows from each expert's output and scatter to the
    original routed order.
"""

from functools import lru_cache

import numpy as np

import concourse.bacc as bacc
import concourse.tile as tile
from concourse import mybir
from concourse import bass_utils

E = 8
K = 2
NTOK = 2048
D = 1024
DFF = 1024
ALIGN = 16
N_CORES = 8

DB = D // 128     # 8 contraction blocks for fc1
JB = DFF // 128   # 8 contraction blocks for fc2
NPAIR = DFF // 128  # 8 GLU pairs (y_p, gate_p), each 128 wide

F32 = mybir.dt.float32
F32R = mybir.dt.float32r


def _plan_chunks(c: int) -> tuple[int, ...]:
    """Split padded capacity c into equal token chunks <=512 (fp32 moving-
    operand max / one PSUM bank), multiples of 8. float32r runs at full PE
    rate only for chunks >=256, which holds whenever c >= 512."""
    n = max(1, -(-c // 512))
    base = -(-c // (8 * n)) * 8
    return (base,) * n


@lru_cache(maxsize=4)
def _build(c_pad: int, chunks: tuple[int, ...]):
    nc = bacc.Bacc("TRN2", target_bir_lowering=False, debug=False)

    # All inputs pre-packed on host to partition-major SBUF layout:
    #   x_pack[p, db*c_pad + t]          = x_e.T[db*128+p, t]
    #   w1_pack[p, (pr*DB + db)*256 + f] = w1p.T[db*128+p, 256*pr + f]
    #   w2_pack[p, jb*D + m]             = fc2[e].T[jb*128+p, m]
    #   out_pack[p, mb*c_pad + t]        = out_e.T[mb*128+p, t]
    x_t = nc.dram_tensor("x_pack", [128, DB * c_pad], F32R, kind="ExternalInput")
    w1_t = nc.dram_tensor("w1_pack", [128, NPAIR * DB * 256], F32R,
                          kind="ExternalInput")
    w2_t = nc.dram_tensor("w2_pack", [128, JB * D], F32R, kind="ExternalInput")
    out_t = nc.dram_tensor("out_pack", [128, DB * c_pad], F32,
                           kind="ExternalOutput")

    with tile.TileContext(nc) as tc:
        with (
            tc.tile_pool(name="xw", bufs=1) as xw,
            tc.tile_pool(name="hps", bufs=3, space="PSUM") as hps,
            tc.tile_pool(name="ops", bufs=2, space="PSUM") as ops,
            tc.tile_pool(name="act", bufs=4) as actp,
            tc.tile_pool(name="apool", bufs=1) as apool,
            tc.tile_pool(name="outp", bufs=2) as outp,
        ):
            # DMA order = dependency order of the PE stream: x first, then
            # fc1 weights pair by pair, then fc2 weights.
            x_sb = xw.tile([128, DB * c_pad], F32R, name="x_sb")
            nc.sync.dma_start(out=x_sb, in_=x_t[:, :])

            w1_sb = []
            for p in range(NPAIR):
                t = xw.tile([128, DB * 256], F32R, name=f"w1_{p}")
                nc.sync.dma_start(
                    out=t, in_=w1_t[:, p * DB * 256:(p + 1) * DB * 256])
                w1_sb.append(t)

            w2_sb = xw.tile([128, JB * D], F32R, name="w2_sb")
            nc.sync.dma_start(out=w2_sb, in_=w2_t[:, :])

            # fc1 + gated activation, both token chunks.
            a_sb = {}
            off = 0
            for ci, w in enumerate(chunks):
                for p in range(NPAIR):
                    y_ps = hps.tile([128, w], F32, tag="y", name=f"y_{ci}_{p}")
                    g_ps = hps.tile([128, w], F32, tag="g", name=f"g_{ci}_{p}")
                    for b in range(DB):
                        rhs = x_sb[:, b * c_pad + off:b * c_pad + off + w]
                        lhs = w1_sb[p][:, b * 256:b * 256 + 128]
                        nc.tensor.matmul(y_ps, lhs, rhs,
                                         start=(b == 0), stop=(b == DB - 1))
                    for b in range(DB):
                        rhs = x_sb[:, b * c_pad + off:b * c_pad + off + w]
                        lhs = w1_sb[p][:, b * 256 + 128:b * 256 + 256]
                        nc.tensor.matmul(g_ps, lhs, rhs,
                                         start=(b == 0), stop=(b == DB - 1))
                    silu = actp.tile([128, w], F32, tag="silu", name=f"s_{ci}_{p}")
                    nc.scalar.activation(
                        out=silu, in_=g_ps,
                        func=mybir.ActivationFunctionType.Silu,
                    )
                    a = apool.tile([128, w], F32R, tag=f"a{ci}_{p}",
                                   name=f"a_{ci}_{p}")
                    nc.vector.tensor_mul(a, y_ps, silu)
                    a_sb[ci, p] = a
                off += w

            # fc2, per chunk; results staged into one SBUF tile per chunk and
            # shipped with a single strided DMA.
            off = 0
            for ci, w in enumerate(chunks):
                o_sb = outp.tile([128, DB * w], F32, tag="osb", name=f"ob_{ci}")
                for mb in range(DB):
                    o_ps = ops.tile([128, w], F32, tag="o", name=f"o_{ci}_{mb}")
                    for b in range(JB):
                        nc.tensor.matmul(
                            o_ps,
                            w2_sb[:, b * D + mb * 128:b * D + (mb + 1) * 128],
                            a_sb[ci, b],
                            start=(b == 0), stop=(b == JB - 1),
                        )
                    nc.vector.tensor_copy(o_sb[:, mb * w:(mb + 1) * w], o_ps)
                out3 = out_t.rearrange("p (mb t) -> p mb t", mb=DB)
                nc.sync.dma_start(
                    out=out3[:, :, off:off + w],
                    in_=o_sb.rearrange("p (mb t) -> p mb t", mb=DB),
                )
                off += w

    nc.compile()
    return nc


def _route(indices: np.ndarray, counts: np.ndarray):
    """Reference's sort/rank math: stable sort by expert, rank within group."""
    n = indices.size
    flat = indices.reshape(-1).astype(np.int64)
    order = np.argsort(flat, kind="stable")
    exp_sorted = flat[order]
    counts64 = counts.astype(np.int64)
    u_start = np.cumsum(counts64) - counts64
    rank = np.arange(n, dtype=np.int64) - u_start[exp_sorted]
    return order, exp_sorted, rank, counts64


def _pack_inputs(x, fc1_weight, fc2_weight, e, sel, r, tok_sel, c_pad):
    """Build core e's packed inputs (see _build docstring for layouts)."""
    xe = np.zeros((c_pad, D), np.float32)
    xe[r] = x[tok_sel]
    # (c_pad, D) -> xT (D, c_pad) -> [db, 128, c] -> (128, db, c)
    x_pack = np.ascontiguousarray(
        xe.T.reshape(DB, 128, c_pad).transpose(1, 0, 2).reshape(128, DB * c_pad)
    )
    # fc1[e]: (2*DFF, D); pair-interleave rows: (y_p, gate_p) adjacent.
    # w1_pack[p, (pr*DB+db)*256 + s*128 + f] = fc1[e][s*DFF + pr*128 + f,
    #                                                 db*128 + p]
    w1 = fc1_weight[e].reshape(2, NPAIR, 128, DB, 128)
    # dims: (s, pr, f, db, p) -> (p, pr, db, s, f)
    w1_pack = np.ascontiguousarray(
        w1.transpose(4, 1, 3, 0, 2).reshape(128, NPAIR * DB * 256)
    )
    # fc2[e]: (D, DFF); w2_pack[p, jb*D + m] = fc2[e][m, jb*128 + p]
    w2 = fc2_weight[e].reshape(D, JB, 128)  # (m, jb, p)
    w2_pack = np.ascontiguousarray(
        w2.transpose(2, 1, 0).reshape(128, JB * D)
    )
    return {"x_pack": x_pack, "w1_pack": w1_pack, "w2_pack": w2_pack}


def _prepare(x, fc1_weight, fc2_weight, indices, counts):
    """Route on host, build/fetch the compiled kernel, pack per-core inputs."""
    x = np.ascontiguousarray(np.asarray(x, dtype=np.float32))
    fc1_weight = np.asarray(fc1_weight, dtype=np.float32)
    fc2_weight = np.asarray(fc2_weight, dtype=np.float32)
    indices = np.asarray(indices)
    counts = np.asarray(counts)

    order, exp_sorted, rank, counts64 = _route(indices, counts)
    tok = order // K

    padded = ((counts64 + ALIGN - 1) // ALIGN) * ALIGN
    c = max(int(padded.max()), ALIGN)
    chunks = _plan_chunks(c)
    c_pad = int(sum(chunks))

    nc = _build(c_pad, chunks)

    in_maps = []
    sels = []
    for e in range(E):
        sel = exp_sorted == e
        sels.append(sel)
        in_maps.append(
            _pack_inputs(x, fc1_weight, fc2_weight, e, sel, rank[sel],
                         tok[sel], c_pad)
        )
    return nc, in_maps, (order, sels, rank, c_pad, indices.size)


def _unpack_outputs(results, meta):
    order, sels, rank, c_pad, n = meta
    out = np.zeros((n, D), np.float32)
    for e in range(E):
        sel = sels[e]
        # out_pack (128, DB*c_pad) -> outT (D, c_pad) -> (c_pad, D)
        op = results[e]["out_pack"].reshape(128, DB, c_pad)
        oe = op.transpose(1, 0, 2).reshape(D, c_pad)
        out[order[sel]] = oe.T[rank[sel]]
    return out


def kernel(x, fc1_weight, fc2_weight, indices, counts):
    nc, in_maps, meta = _prepare(x, fc1_weight, fc2_weight, indices, counts)
    res = bass_utils.run_bass_kernel_spmd(nc, in_maps, core_ids=list(range(N_CORES)))
    return _unpack_outputs(res.results, meta)


# revision 6
# speedup vs baseline: 1.3037x; 1.0367x over previous
"""Expert-parallel MoE GLU FFN for Trainium2 (8 NeuronCores, Bass/Tile).

Problem: nn_ExpertFFNGroupedMM (E=8 experts, K=2, NTOK=2048, D=1024, DFF=1024,
ALIGN=16). Reference: sort routed rows by expert, pad each expert group to a
multiple of 16, grouped GEMM fc1 (GLU) -> y*silu(gate) -> grouped GEMM fc2,
scatter back to original routed-row order.

Strategy (expert parallelism, per the sharding hint):
  - Host: stable-sort routed rows by expert id, compute per-expert ranks
    (exactly the reference's pad_sorted_idxs math), and build one dense token
    slab per expert, zero-padded to a common static capacity C_pad.
    This is the "all-to-all after sort" done on host.
  - Host pre-packs every device input into its exact SBUF layout
    (partition-major), so each input lands in SBUF with a handful of large
    fully-contiguous DMAs (the naive strided layout left the PE starved and
    HAM-throttled behind ~100 small DMA descriptors).
  - Device (core e): h = fc1[e] @ x (contraction dim on SBUF partitions ->
    transpose-free), a = y * silu(gate), out = fc2[e] @ a. Matmuls run as
    float32r (reduced-mantissa full-rate fp32 PE mode, ~2e-4 rel err).
  - Host: gather rank-rows from each expert's output and scatter to the
    original routed order.
"""

from functools import lru_cache

import numpy as np

import concourse.bacc as bacc
import concourse.tile as tile
from concourse import mybir
from concourse import bass_utils

E = 8
K = 2
NTOK = 2048
D = 1024
DFF = 1024
ALIGN = 16
N_CORES = 8

DB = D // 128     # 8 contraction blocks for fc1
JB = DFF // 128   # 8 contraction blocks for fc2
NPAIR = DFF // 128  # 8 GLU pairs (y_p, gate_p), each 128 wide

F32 = mybir.dt.float32
F32R = mybir.dt.float32r


def _plan_chunks(c: int) -> tuple[int, ...]:
    """Split padded capacity c into equal token chunks <=512 (fp32 moving-
    operand max / one PSUM bank), multiples of 8. float32r runs at full PE
    rate only for chunks >=256, which holds whenever c >= 512."""
    n = max(1, -(-c // 512))
    base = -(-c // (8 * n)) * 8
    return (base,) * n


@lru_cache(maxsize=4)
def _build(c_pad: int, chunks: tuple[int, ...]):
    nc = bacc.Bacc("TRN2", target_bir_lowering=False, debug=False)

    # All inputs pre-packed on host to partition-major SBUF layout:
    #   x_pack[p, db*c_pad + t]          = x_e.T[db*128+p, t]
    #   w1_pack[p, (pr*DB + db)*256 + f] = w1p.T[db*128+p, 256*pr + f]
    #   w2_pack[p, jb*D + m]             = fc2[e].T[jb*128+p, m]
    #   out_pack[p, mb*c_pad + t]        = out_e.T[mb*128+p, t]
    x_t = nc.dram_tensor("x_pack", [128, DB * c_pad], F32R, kind="ExternalInput")
    w1_t = nc.dram_tensor("w1_pack", [128, NPAIR * DB * 256], F32R,
                          kind="ExternalInput")
    w2_t = nc.dram_tensor("w2_pack", [128, JB * D], F32R, kind="ExternalInput")
    out_t = nc.dram_tensor("out_pack", [128, DB * c_pad], F32,
                           kind="ExternalOutput")

    with tile.TileContext(nc) as tc:
        with (
            tc.tile_pool(name="xw", bufs=1) as xw,
            tc.tile_pool(name="hps", bufs=3, space="PSUM") as hps,
            tc.tile_pool(name="ops", bufs=2, space="PSUM") as ops,
            tc.tile_pool(name="act", bufs=4) as actp,
            tc.tile_pool(name="apool", bufs=1) as apool,
            tc.tile_pool(name="outp", bufs=2) as outp,
        ):
            # DMA order = dependency order of the PE stream: first GLU pair's
            # weights and the token slab (per d-block, so the first matmul
            # only waits on ~1.3MB), then remaining fc1 pairs, then fc2.
            w1_sb = []
            for p in range(NPAIR):
                t = xw.tile([128, DB * 256], F32R, name=f"w1_{p}")
                w1_sb.append(t)
            nc.sync.dma_start(out=w1_sb[0], in_=w1_t[:, 0:DB * 256])

            x_sb = []
            for b in range(DB):
                t = xw.tile([128, c_pad], F32R, name=f"x_{b}")
                nc.sync.dma_start(
                    out=t, in_=x_t[:, b * c_pad:(b + 1) * c_pad])
                x_sb.append(t)

            for p in range(1, NPAIR):
                nc.sync.dma_start(
                    out=w1_sb[p], in_=w1_t[:, p * DB * 256:(p + 1) * DB * 256])

            w2_sb = xw.tile([128, JB * D], F32R, name="w2_sb")
            nc.sync.dma_start(out=w2_sb, in_=w2_t[:, :])

            # fc1 + gated activation, both token chunks.
            a_sb = {}
            off = 0
            for ci, w in enumerate(chunks):
                for p in range(NPAIR):
                    y_ps = hps.tile([128, w], F32, tag="y", name=f"y_{ci}_{p}")
                    g_ps = hps.tile([128, w], F32, tag="g", name=f"g_{ci}_{p}")
                    for b in range(DB):
                        rhs = x_sb[b][:, off:off + w]
                        lhs = w1_sb[p][:, b * 256:b * 256 + 128]
                        nc.tensor.matmul(y_ps, lhs, rhs,
                                         start=(b == 0), stop=(b == DB - 1))
                    for b in range(DB):
                        rhs = x_sb[b][:, off:off + w]
                        lhs = w1_sb[p][:, b * 256 + 128:b * 256 + 256]
                        nc.tensor.matmul(g_ps, lhs, rhs,
                                         start=(b == 0), stop=(b == DB - 1))
                    silu = actp.tile([128, w], F32, tag="silu", name=f"s_{ci}_{p}")
                    nc.scalar.activation(
                        out=silu, in_=g_ps,
                        func=mybir.ActivationFunctionType.Silu,
                    )
                    a = apool.tile([128, w], F32R, tag=f"a{ci}_{p}",
                                   name=f"a_{ci}_{p}")
                    nc.vector.tensor_mul(a, y_ps, silu)
                    a_sb[ci, p] = a
                off += w

            # fc2, per chunk; results staged into one SBUF tile per chunk and
            # shipped with a single strided DMA.
            off = 0
            for ci, w in enumerate(chunks):
                o_sb = outp.tile([128, DB * w], F32, tag="osb", name=f"ob_{ci}")
                for mb in range(DB):
                    o_ps = ops.tile([128, w], F32, tag="o", name=f"o_{ci}_{mb}")
                    for b in range(JB):
                        nc.tensor.matmul(
                            o_ps,
                            w2_sb[:, b * D + mb * 128:b * D + (mb + 1) * 128],
                            a_sb[ci, b],
                            start=(b == 0), stop=(b == JB - 1),
                        )
                    nc.vector.tensor_copy(o_sb[:, mb * w:(mb + 1) * w], o_ps)
                    nc.sync.dma_start(
                        out=out_t[:, mb * c_pad + off:mb * c_pad + off + w],
                        in_=o_sb[:, mb * w:(mb + 1) * w],
                    )
                off += w

    nc.compile()
    return nc


def _route(indices: np.ndarray, counts: np.ndarray):
    """Reference's sort/rank math: stable sort by expert, rank within group."""
    n = indices.size
    flat = indices.reshape(-1).astype(np.int64)
    order = np.argsort(flat, kind="stable")
    exp_sorted = flat[order]
    counts64 = counts.astype(np.int64)
    u_start = np.cumsum(counts64) - counts64
    rank = np.arange(n, dtype=np.int64) - u_start[exp_sorted]
    return order, exp_sorted, rank, counts64


def _pack_inputs(x, fc1_weight, fc2_weight, e, sel, r, tok_sel, c_pad):
    """Build core e's packed inputs (see _build docstring for layouts)."""
    xe = np.zeros((c_pad, D), np.float32)
    xe[r] = x[tok_sel]
    # (c_pad, D) -> xT (D, c_pad) -> [db, 128, c] -> (128, db, c)
    x_pack = np.ascontiguousarray(
        xe.T.reshape(DB, 128, c_pad).transpose(1, 0, 2).reshape(128, DB * c_pad)
    )
    # fc1[e]: (2*DFF, D); pair-interleave rows: (y_p, gate_p) adjacent.
    # w1_pack[p, (pr*DB+db)*256 + s*128 + f] = fc1[e][s*DFF + pr*128 + f,
    #                                                 db*128 + p]
    w1 = fc1_weight[e].reshape(2, NPAIR, 128, DB, 128)
    # dims: (s, pr, f, db, p) -> (p, pr, db, s, f)
    w1_pack = np.ascontiguousarray(
        w1.transpose(4, 1, 3, 0, 2).reshape(128, NPAIR * DB * 256)
    )
    # fc2[e]: (D, DFF); w2_pack[p, jb*D + m] = fc2[e][m, jb*128 + p]
    w2 = fc2_weight[e].reshape(D, JB, 128)  # (m, jb, p)
    w2_pack = np.ascontiguousarray(
        w2.transpose(2, 1, 0).reshape(128, JB * D)
    )
    return {"x_pack": x_pack, "w1_pack": w1_pack, "w2_pack": w2_pack}


def _prepare(x, fc1_weight, fc2_weight, indices, counts):
    """Route on host, build/fetch the compiled kernel, pack per-core inputs."""
    x = np.ascontiguousarray(np.asarray(x, dtype=np.float32))
    fc1_weight = np.asarray(fc1_weight, dtype=np.float32)
    fc2_weight = np.asarray(fc2_weight, dtype=np.float32)
    indices = np.asarray(indices)
    counts = np.asarray(counts)

    order, exp_sorted, rank, counts64 = _route(indices, counts)
    tok = order // K

    padded = ((counts64 + ALIGN - 1) // ALIGN) * ALIGN
    c = max(int(padded.max()), ALIGN)
    chunks = _plan_chunks(c)
    c_pad = int(sum(chunks))

    nc = _build(c_pad, chunks)

    in_maps = []
    sels = []
    for e in range(E):
        sel = exp_sorted == e
        sels.append(sel)
        in_maps.append(
            _pack_inputs(x, fc1_weight, fc2_weight, e, sel, rank[sel],
                         tok[sel], c_pad)
        )
    return nc, in_maps, (order, sels, rank, c_pad, indices.size)


def _unpack_outputs(results, meta):
    order, sels, rank, c_pad, n = meta
    out = np.zeros((n, D), np.float32)
    for e in range(E):
        sel = sels[e]
        # out_pack (128, DB*c_pad) -> outT (D, c_pad) -> (c_pad, D)
        op = results[e]["out_pack"].reshape(128, DB, c_pad)
        oe = op.transpose(1, 0, 2).reshape(D, c_pad)
        out[order[sel]] = oe.T[rank[sel]]
    return out


def kernel(x, fc1_weight, fc2_weight, indices, counts):
    nc, in_maps, meta = _prepare(x, fc1_weight, fc2_weight, indices, counts)
    res = bass_utils.run_bass_kernel_spmd(nc, in_maps, core_ids=list(range(N_CORES)))
    return _unpack_outputs(res.results, meta)
